# revision 5
# baseline (speedup 1.0000x reference)
"""Trainium2 Bass kernel for DenseGraphSimpleOpEdgeFlow (GNN message passing).

Reference semantics (per batch b):
  support = inputs @ weight                                    [N, F]
  op_emb[diag] = self_op_emb; adjP = adj + I
  attn = sigmoid(op_emb @ attn_w + attn_b)                     [N, N, F]
  attn = (adjP != 0) * attn;  attn = 1 where adjP == 1 (skip)
  out[i, :] = sum_j attn[i, j, :] * support[j, :] + support[i, :]

Sharding: data-parallel over batch B=64 across 8 cores (8 batches/core).

v3 device mapping (per core, 8 batches = 16 "halves" of 48 i-rows each):
  * z = W_aug^T @ op_emb^T on TensorE in bf16, K=49: rows 0-47 = attn_w,
    row 48 = all-ones paired with a mask-logit row m2 in {0,-100} in the
    moving tensor (drives sigmoid of masked/skip edges to 0). One matmul
    instruction per 1536-col PSUM tile (free=1536) to amortize LDWEIGHTS
    and the PE SBUF access latency over 3x more columns.
  * sigmoid on ScalarE straight out of PSUM (1536-col chunks), attn_b as
    per-partition bias, bf16 output. ScalarE does NOTHING else.
  * contraction sum_j sigma[d,(i,j)] * support_T[d,j]: DVE bf16 2x for
    the multiply + tree level L1 + the final 6->1 reduce; Pool (gpsimd)
    for tree levels L2,L3,L4. One bulk +S1 add at the end on DVE.
  * skip/identity term S1[d,(b,i)] = sum_j support[j,d]*(skipmask+I)[j,i]
    as small TensorE matmuls per batch (mask built on device from adj).
  * output transpose [d,(b,i)] -> [(b,i),d] on TensorE, copies on DVE.
"""

import numpy as np

B, N, IN_F, OUT_F, OP_D = 64, 96, 128, 128, 48
NCORES = 8
BPC = B // NCORES  # batches per core
NIH = N // 2       # 48 i-rows per half
HALF = NIH * N     # 4608 columns per half
NEG = -100.0
MM_FREE = 512      # columns per matmul instruction

_CACHE = {}


def _build_nc():
    import concourse.bass as bass
    import concourse.bacc as bacc
    import concourse.tile as tile
    from concourse import mybir
    from contextlib import ExitStack

    f32 = mybir.dt.float32
    bf16 = mybir.dt.bfloat16
    i32 = mybir.dt.int32
    MUL = mybir.AluOpType.mult
    ADD = mybir.AluOpType.add
    ISEQ = mybir.AluOpType.is_equal
    SIG = mybir.ActivationFunctionType.Sigmoid

    nc = bacc.Bacc(None, target_bir_lowering=False)

    # ---- DRAM parameters ----
    # op4[b, pb] is [49, HALF] bf16: rows 0-47 = op_emb^T, row 48 = m2.
    p_op4 = nc.declare_dram_parameter("op4", [BPC, 2, OP_D + 1, HALF], bf16,
                                      isOutput=False)
    p_adjs = nc.declare_dram_parameter("adjs", [N, BPC, N], i32, isOutput=False)  # [j, b, i]
    # packed constants: pbf = [w2 | wgt | inpt], pf32 = [ident | attnb | thr | eye]
    p_pbf = nc.declare_dram_parameter("pbf", [128, 1024], bf16, isOutput=False)
    p_pf32 = nc.declare_dram_parameter("pf32", [128, 321], f32, isOutput=False)
    p_out = nc.declare_dram_parameter("out", [BPC * N, OUT_F], f32, isOutput=True)

    NB = BPC * N  # 768

    def bcast_b(ap):
        return bass.AP(tensor=ap.tensor, offset=ap.offset,
                       ap=[ap.ap[0], [0, BPC], ap.ap[1]])

    def sub_ap(ap, extra_off, dims):
        return bass.AP(tensor=ap.tensor, offset=ap.offset + extra_off,
                       ap=[ap.ap[0]] + dims)

    with tile.TileContext(nc) as tc, ExitStack() as ctx:
        const = ctx.enter_context(tc.tile_pool(name="const", bufs=1))
        rhs_pool = ctx.enter_context(tc.tile_pool(name="rhs", bufs=8))

        pbf_sb = const.tile([128, 1024], bf16)
        nc.gpsimd.dma_start(out=pbf_sb[:], in_=p_pbf[:, :])
        pf32_sb = const.tile([128, 321], f32)
        nc.gpsimd.dma_start(out=pf32_sb[:], in_=p_pf32[:, :])
        adjs_sb0 = const.tile([N, BPC, N], i32)
        nc.gpsimd.dma_start(out=adjs_sb0[:], in_=p_adjs[:, :, :])

        w2_sb = pbf_sb[:, 0:128]
        wgt_sb = pbf_sb[:, 128:256]
        inpt_sb = pbf_sb[:, 256:1024]
        ident_sb = pf32_sb[:, 0:128]
        attnb_sb = pf32_sb[:, 128:129]
        thr_sb = pf32_sb[0:N, 129:225]
        eye_sb = pf32_sb[0:N, 225:321]

        # sigmoid ACT-table warm
        warm_sb = const.tile([OUT_F, 1], bf16)
        nc.scalar.activation(out=warm_sb[:], in_=attnb_sb, func=SIG)

        # bf16 moving-tensor loads: one tile per batch, pb0 data at
        # partitions 0-48, pb1 at 64-112.
        rts = [None] * BPC

        def load_rt(b):
            rt = rhs_pool.tile([128, HALF], bf16, tag="rt")
            for pbi, pb in enumerate((0, 64)):
                nc.gpsimd.dma_start(out=rt[pb:pb + OP_D + 1, :], in_=p_op4[b, pbi])
            rts[b] = rt

        for b in range(BPC):
            load_rt(b)

        stbf_sb = const.tile([OUT_F, NB], bf16)   # support^T in bf16
        s1_sb = const.tile([OUT_F, NB], f32)      # skip+identity term
        snat_sb = const.tile([N, BPC, OUT_F], bf16)  # support natural [j, b, d]
        out_fin = const.tile([OUT_F, NB], f32)

        pz = ctx.enter_context(tc.tile_pool(name="pz", bufs=2, space="PSUM"))
        ptr = ctx.enter_context(tc.tile_pool(name="ptr", bufs=2, space="PSUM"))

        # ---------------- pre-phase: support, skip mask ----------------
        psb = ctx.enter_context(tc.tile_pool(name="pre_sb", bufs=1))
        for c0, cw in ((0, 512), (512, 256)):
            stp = ptr.tile([128, 512], f32, tag="pt")
            nc.tensor.matmul(stp[:, 0:cw], lhsT=wgt_sb,
                             rhs=inpt_sb[:, c0:c0 + cw], start=True, stop=True)
            nc.vector.tensor_copy(out=stbf_sb[:, c0:c0 + cw], in_=stp[:, 0:cw])

        for b in range(BPC):
            pn = ptr.tile([128, 512], f32, tag="pt")
            nc.tensor.matmul(pn[:N, 0:OUT_F], lhsT=inpt_sb[:, b * N:(b + 1) * N],
                             rhs=wgt_sb, start=True, stop=True)
            nc.vector.tensor_copy(out=snat_sb[:, b, :], in_=pn[:N, 0:OUT_F])

        skf = psb.tile([N, BPC, N], f32, tag="skf")
        nc.gpsimd.tensor_copy(out=skf[:], in_=adjs_sb0[:])
        sk1 = psb.tile([N, BPC, N], f32, tag="sk1")
        nc.vector.tensor_tensor(out=sk1[:], in0=skf[:], in1=bcast_b(thr_sb), op=ISEQ)
        skim = psb.tile([N, BPC, N], bf16, tag="skim")
        nc.gpsimd.tensor_tensor(out=skim[:], in0=sk1[:], in1=bcast_b(eye_sb), op=ADD)

        for b in range(BPC):
            ps1 = ptr.tile([128, 512], f32, tag="pt")
            nc.tensor.matmul(ps1[:, 0:N], lhsT=snat_sb[:, b, :],
                             rhs=skim[:, b, :], start=True, stop=True)
            nc.vector.tensor_copy(out=s1_sb[:, b * N:(b + 1) * N], in_=ps1[:, 0:N])

        # ---------------- main loop: 16 halves ----------------
        sig_pool = ctx.enter_context(tc.tile_pool(name="sig", bufs=2))
        prod_pool = ctx.enter_context(tc.tile_pool(name="prod", bufs=2))
        l1_pool = ctx.enter_context(tc.tile_pool(name="l1", bufs=2))
        l2_pool = ctx.enter_context(tc.tile_pool(name="l2", bufs=2))
        l4_pool = ctx.enter_context(tc.tile_pool(name="l4", bufs=2))

        for pbi, pb in enumerate((0, 64)):
            for b in range(BPC):
                rt = rts[b]
                sig_t = sig_pool.tile([OUT_F, HALF], bf16)
                for c in range(3):
                    pzt = pz.tile([OUT_F, 1536], f32, tag="z")
                    co = c * 1536
                    for m0 in range(0, 1536, MM_FREE):
                        mw = min(MM_FREE, 1536 - m0)
                        nc.tensor.matmul(
                            pzt[:, m0:m0 + mw],
                            lhsT=w2_sb[pb:pb + OP_D + 1, :],
                            rhs=rt[pb:pb + OP_D + 1, co + m0:co + m0 + mw],
                            start=True, stop=True)
                    nc.scalar.activation(out=sig_t[:, co:co + 1536],
                                         in_=pzt[:], func=SIG,
                                         bias=attnb_sb, scale=1.0)

                st_b = stbf_sb[:, b * N:(b + 1) * N]
                st_bcast = bass.AP(tensor=st_b.tensor, offset=st_b.offset,
                                   ap=[st_b.ap[0], [0, NIH], st_b.ap[1]])
                prod = prod_pool.tile([OUT_F, HALF], bf16)
                nc.vector.tensor_tensor(out=prod[:], in0=sig_t[:],
                                        in1=st_bcast, op=MUL)
                # tree: 96 -> 48 (DVE) -> 24 (Pool) -> 12 (Pool) -> 6 (Pool) -> 1 (DVE)
                l1 = l1_pool.tile([OUT_F, NIH * 48], bf16)
                nc.vector.tensor_tensor(
                    out=l1[:],
                    in0=sub_ap(prod[:], 0, [[96, NIH], [1, 48]]),
                    in1=sub_ap(prod[:], 48, [[96, NIH], [1, 48]]), op=ADD)
                l2 = l2_pool.tile([OUT_F, NIH * 24], bf16)
                nc.gpsimd.tensor_tensor(
                    out=l2[:],
                    in0=sub_ap(l1[:], 0, [[48, NIH], [1, 24]]),
                    in1=sub_ap(l1[:], 24, [[48, NIH], [1, 24]]), op=ADD)
                l3 = l2_pool.tile([OUT_F, NIH * 12], bf16, tag="l3")
                nc.gpsimd.tensor_tensor(
                    out=l3[:],
                    in0=sub_ap(l2[:], 0, [[24, NIH], [1, 12]]),
                    in1=sub_ap(l2[:], 12, [[24, NIH], [1, 12]]), op=ADD)
                l4 = l4_pool.tile([OUT_F, NIH * 6], bf16)
                nc.gpsimd.tensor_tensor(
                    out=l4[:],
                    in0=sub_ap(l3[:], 0, [[12, NIH], [1, 6]]),
                    in1=sub_ap(l3[:], 6, [[12, NIH], [1, 6]]), op=ADD)
                cb = b * N + pbi * NIH
                nc.vector.tensor_reduce(out=out_fin[:, cb:cb + NIH],
                                        in_=sub_ap(l4[:], 0, [[6, NIH], [1, 6]]),
                                        axis=mybir.AxisListType.X, op=ADD)

        # one bulk skip/identity add over all batches
        nc.vector.tensor_tensor(out=out_fin[:], in0=out_fin[:], in1=s1_sb[:],
                                op=ADD)

        # ---------------- output transpose + store ----------------
        outp = ctx.enter_context(tc.tile_pool(name="outp", bufs=2))
        for c in range(6):
            pt = ptr.tile([128, 512], f32, tag="pt")
            nc.tensor.transpose(pt[:, 0:128], out_fin[:, c * 128:(c + 1) * 128],
                                ident_sb)
            ot = outp.tile([128, 128], f32)
            nc.vector.tensor_copy(out=ot[:], in_=pt[:, 0:128])
            nc.sync.dma_start(out=p_out[c * 128:(c + 1) * 128, :], in_=ot[:])

    nc.finalize()
    return nc


def _get_nc():
    if "nc" not in _CACHE:
        _CACHE["nc"] = _build_nc()
    return _CACHE["nc"]


def marshal_core(inputs, adj, op_emb, weight, attn_w, attn_b, self_op_emb, core):
    """Build the in_map for one core (layout/dtype marshaling + mask logits)."""
    import ml_dtypes
    bfloat16 = ml_dtypes.bfloat16

    sl = slice(core * BPC, (core + 1) * BPC)
    op_sh = np.array(op_emb[sl], np.float32)              # [BPC, N, N, OP_D]
    idx = np.arange(N)
    op_sh[:, idx, idx, :] = np.asarray(self_op_emb, np.float32)
    op_t = op_sh.transpose(0, 3, 1, 2)                    # [BPC, OP_D, N(i), N(j)]
    adj_sh = np.asarray(adj[sl]).astype(np.int32)         # [BPC, N, N]
    eye = np.eye(N, dtype=np.float32)
    adjp = adj_sh.astype(np.float32) + eye
    m2 = np.where(adjp <= 1.0, np.float32(NEG), np.float32(0.0))  # [BPC, N, N]

    op4 = np.empty((BPC, 2, OP_D + 1, HALF), bfloat16)
    op4[:, :, :OP_D, :] = op_t.reshape(BPC, OP_D, 2, HALF).transpose(
        0, 2, 1, 3).astype(bfloat16)
    op4[:, :, OP_D, :] = m2.reshape(BPC, 2, HALF).astype(bfloat16)

    adjs = np.ascontiguousarray(adj_sh.transpose(2, 0, 1))  # [j, b, i]
    inpt = np.ascontiguousarray(
        np.asarray(inputs[sl], np.float32).reshape(BPC * N, IN_F).T)

    w2 = np.zeros((128, 128), np.float32)
    w2[0:OP_D] = attn_w
    w2[OP_D] = 1.0
    w2[64:64 + OP_D] = attn_w
    w2[64 + OP_D] = 1.0

    pbf = np.zeros((128, 1024), bfloat16)
    pbf[:, 0:128] = w2.astype(bfloat16)
    pbf[:, 128:256] = np.asarray(weight, np.float32).astype(bfloat16)
    pbf[:, 256:1024] = inpt.astype(bfloat16)
    pf32 = np.zeros((128, 321), np.float32)
    pf32[:, 0:128] = np.eye(128, dtype=np.float32)
    pf32[:, 128] = np.asarray(attn_b, np.float32)
    pf32[0:N, 129:225] = 1.0 - eye
    pf32[0:N, 225:321] = eye

    return {
        "op4": op4,
        "adjs": adjs,
        "pbf": pbf,
        "pf32": pf32,
    }


def _ensure_ntff_hook():
    """Provide antenv.axon_hooks if the image lacks it (NTFF timing under axon)."""
    import sys as _sys

    try:
        from antenv.axon_hooks import get_axon_ntff_profile_hook  # noqa: F401
        return
    except ImportError:
        pass

    import contextlib
    import ctypes
    import types

    so_path = "/opt/axon/libaxon_pjrt.so"
    try:
        lib = ctypes.CDLL(so_path)
    except OSError:
        lib = None
    if lib is None or not hasattr(lib, "axon_start_nrt_profile"):
        hook = None
    else:
        lib.axon_start_nrt_profile.argtypes = [
            ctypes.POINTER(ctypes.c_int64), ctypes.c_size_t]
        lib.axon_start_nrt_profile.restype = ctypes.c_int64
        lib.axon_stop_nrt_profile.argtypes = [ctypes.c_char_p]
        lib.axon_stop_nrt_profile.restype = ctypes.c_int64

        @contextlib.contextmanager
        def hook(output_dir, device_ids):
            import jax
            jax.devices()
            if device_ids:
                ids = (ctypes.c_int64 * len(device_ids))(*device_ids)
                rc = lib.axon_start_nrt_profile(ids, len(device_ids))
            else:
                rc = lib.axon_start_nrt_profile(None, 0)
            if rc != 0:
                raise RuntimeError(f"axon_start_nrt_profile rc={rc}")
            try:
                yield
            finally:
                n = lib.axon_stop_nrt_profile(str(output_dir).encode())
                print(f"ntff profile: {n} file(s) written to {output_dir}")

    mod = types.ModuleType("antenv.axon_hooks")
    _state = {"hook": hook}
    mod.get_axon_ntff_profile_hook = lambda: _state["hook"]

    def _set(h):
        _state["hook"] = h

    mod.set_axon_ntff_profile_hook = _set
    _sys.modules["antenv.axon_hooks"] = mod


def run(inputs, adj, op_emb, weight, attn_w, attn_b, self_op_emb, trace=False):
    if trace:
        _ensure_ntff_hook()
    from concourse.bass_utils import run_bass_kernel_spmd

    nc = _get_nc()
    in_maps = [
        marshal_core(inputs, adj, op_emb, weight, attn_w, attn_b, self_op_emb, c)
        for c in range(NCORES)
    ]
    res = run_bass_kernel_spmd(nc, in_maps, core_ids=list(range(NCORES)), trace=trace)
    out = np.concatenate(
        [res.results[c]["out"].reshape(BPC, N, OUT_F) for c in range(NCORES)], axis=0)
    return np.ascontiguousarray(out, np.float32), res


def kernel(inputs, adj, op_emb, weight, attn_w, attn_b, self_op_emb):
    out, _ = run(inputs, adj, op_emb, weight, attn_w, attn_b, self_op_emb, trace=False)
    return out


# revision 6
# speedup vs baseline: 1.0984x; 1.0984x over previous
"""Trainium2 Bass kernel for DenseGraphSimpleOpEdgeFlow (GNN message passing).

Reference semantics (per batch b):
  support = inputs @ weight                                    [N, F]
  op_emb[diag] = self_op_emb; adjP = adj + I
  attn = sigmoid(op_emb @ attn_w + attn_b)                     [N, N, F]
  attn = (adjP != 0) * attn;  attn = 1 where adjP == 1 (skip)
  out[i, :] = sum_j attn[i, j, :] * support[j, :] + support[i, :]

Sharding: data-parallel over batch B=64 across 8 cores (8 batches/core).

v3 device mapping (per core, 8 batches = 16 "halves" of 48 i-rows each):
  * z = W_aug^T @ op_emb^T on TensorE in bf16, K=49: rows 0-47 = attn_w,
    row 48 = all-ones paired with a mask-logit row m2 in {0,-100} in the
    moving tensor (drives sigmoid of masked/skip edges to 0). One matmul
    instruction per 1536-col PSUM tile (free=1536) to amortize LDWEIGHTS
    and the PE SBUF access latency over 3x more columns.
  * sigmoid on ScalarE straight out of PSUM (1536-col chunks), attn_b as
    per-partition bias, bf16 output. ScalarE does NOTHING else.
  * contraction sum_j sigma[d,(i,j)] * support_T[d,j]: DVE bf16 2x for
    the multiply + tree level L1 + the final 6->1 reduce; Pool (gpsimd)
    for tree levels L2,L3,L4. One bulk +S1 add at the end on DVE.
  * skip/identity term S1[d,(b,i)] = sum_j support[j,d]*(skipmask+I)[j,i]
    as small TensorE matmuls per batch (mask built on device from adj).
  * output transpose [d,(b,i)] -> [(b,i),d] on TensorE, copies on DVE.
"""

import numpy as np

B, N, IN_F, OUT_F, OP_D = 64, 96, 128, 128, 48
NCORES = 8
BPC = B // NCORES  # batches per core
NIH = N // 2       # 48 i-rows per half
HALF = NIH * N     # 4608 columns per half
NEG = -100.0
MM_FREE = 512      # columns per matmul instruction

_CACHE = {}


def _build_nc():
    import concourse.bass as bass
    import concourse.bacc as bacc
    import concourse.tile as tile
    from concourse import mybir
    from contextlib import ExitStack

    f32 = mybir.dt.float32
    bf16 = mybir.dt.bfloat16
    i32 = mybir.dt.int32
    MUL = mybir.AluOpType.mult
    ADD = mybir.AluOpType.add
    ISEQ = mybir.AluOpType.is_equal
    SIG = mybir.ActivationFunctionType.Sigmoid

    nc = bacc.Bacc(None, target_bir_lowering=False)

    # ---- DRAM parameters ----
    # op4[b, pb] is [49, HALF] bf16: rows 0-47 = op_emb^T, row 48 = m2.
    p_op4 = nc.declare_dram_parameter("op4", [BPC, 2, OP_D + 1, HALF], bf16,
                                      isOutput=False)
    p_adjs = nc.declare_dram_parameter("adjs", [N, BPC, N], i32, isOutput=False)  # [j, b, i]
    # packed constants: pbf = [w2 | wgt | inpt], pf32 = [ident | attnb | thr | eye]
    p_pbf = nc.declare_dram_parameter("pbf", [128, 1024], bf16, isOutput=False)
    p_pf32 = nc.declare_dram_parameter("pf32", [128, 321], f32, isOutput=False)
    p_out = nc.declare_dram_parameter("out", [BPC * N, OUT_F], f32, isOutput=True)

    NB = BPC * N  # 768

    def bcast_b(ap):
        return bass.AP(tensor=ap.tensor, offset=ap.offset,
                       ap=[ap.ap[0], [0, BPC], ap.ap[1]])

    def sub_ap(ap, extra_off, dims):
        return bass.AP(tensor=ap.tensor, offset=ap.offset + extra_off,
                       ap=[ap.ap[0]] + dims)

    with tile.TileContext(nc) as tc, ExitStack() as ctx:
        const = ctx.enter_context(tc.tile_pool(name="const", bufs=1))
        rhs_pool = ctx.enter_context(tc.tile_pool(name="rhs", bufs=8))

        pbf_sb = const.tile([128, 1024], bf16)
        nc.gpsimd.dma_start(out=pbf_sb[:], in_=p_pbf[:, :])
        pf32_sb = const.tile([128, 321], f32)
        nc.gpsimd.dma_start(out=pf32_sb[:], in_=p_pf32[:, :])
        adjs_sb0 = const.tile([N, BPC, N], i32)
        nc.gpsimd.dma_start(out=adjs_sb0[:], in_=p_adjs[:, :, :])

        w2_sb = pbf_sb[:, 0:128]
        wgt_sb = pbf_sb[:, 128:256]
        inpt_sb = pbf_sb[:, 256:1024]
        ident_sb = pf32_sb[:, 0:128]
        attnb_sb = pf32_sb[:, 128:129]
        thr_sb = pf32_sb[0:N, 129:225]
        eye_sb = pf32_sb[0:N, 225:321]

        # sigmoid ACT-table warm
        warm_sb = const.tile([OUT_F, 1], bf16)
        nc.scalar.activation(out=warm_sb[:], in_=attnb_sb, func=SIG)

        # bf16 moving-tensor loads: one tile per batch, pb0 data at
        # partitions 0-48, pb1 at 64-112.
        rts = [None] * BPC

        def load_rt(b):
            rt = rhs_pool.tile([128, HALF], bf16, tag="rt")
            for pbi, pb in enumerate((0, 64)):
                nc.gpsimd.dma_start(out=rt[pb:pb + OP_D + 1, :], in_=p_op4[b, pbi])
            rts[b] = rt

        for b in range(BPC):
            load_rt(b)

        stbf_sb = const.tile([OUT_F, NB], bf16)   # support^T in bf16
        s1_sb = const.tile([OUT_F, NB], f32)      # skip+identity term
        snat_sb = const.tile([N, BPC, OUT_F], bf16)  # support natural [j, b, d]
        out_fin = const.tile([OUT_F, NB], f32)

        pz = ctx.enter_context(tc.tile_pool(name="pz", bufs=2, space="PSUM"))
        ptr = ctx.enter_context(tc.tile_pool(name="ptr", bufs=2, space="PSUM"))

        # ---------------- pre-phase: support, skip mask ----------------
        psb = ctx.enter_context(tc.tile_pool(name="pre_sb", bufs=1))
        for c0, cw in ((0, 512), (512, 256)):
            stp = ptr.tile([128, 512], f32, tag="pt")
            nc.tensor.matmul(stp[:, 0:cw], lhsT=wgt_sb,
                             rhs=inpt_sb[:, c0:c0 + cw], start=True, stop=True)
            nc.vector.tensor_copy(out=stbf_sb[:, c0:c0 + cw], in_=stp[:, 0:cw])

        for b in range(BPC):
            pn = ptr.tile([128, 512], f32, tag="pt")
            nc.tensor.matmul(pn[:N, 0:OUT_F], lhsT=inpt_sb[:, b * N:(b + 1) * N],
                             rhs=wgt_sb, start=True, stop=True)
            nc.vector.tensor_copy(out=snat_sb[:, b, :], in_=pn[:N, 0:OUT_F])

        skf = psb.tile([N, BPC, N], f32, tag="skf")
        nc.gpsimd.tensor_copy(out=skf[:], in_=adjs_sb0[:])
        sk1 = psb.tile([N, BPC, N], f32, tag="sk1")
        nc.vector.tensor_tensor(out=sk1[:], in0=skf[:], in1=bcast_b(thr_sb), op=ISEQ)
        skim = psb.tile([N, BPC, N], bf16, tag="skim")
        nc.gpsimd.tensor_tensor(out=skim[:], in0=sk1[:], in1=bcast_b(eye_sb), op=ADD)

        for b in range(BPC):
            ps1 = ptr.tile([128, 512], f32, tag="pt")
            nc.tensor.matmul(ps1[:, 0:N], lhsT=snat_sb[:, b, :],
                             rhs=skim[:, b, :], start=True, stop=True)
            nc.vector.tensor_copy(out=s1_sb[:, b * N:(b + 1) * N], in_=ps1[:, 0:N])

        # ---------------- main loop: 16 halves ----------------
        sig_pool = ctx.enter_context(tc.tile_pool(name="sig", bufs=2))
        prod_pool = ctx.enter_context(tc.tile_pool(name="prod", bufs=2))
        l1_pool = ctx.enter_context(tc.tile_pool(name="l1", bufs=2))
        l2_pool = ctx.enter_context(tc.tile_pool(name="l2", bufs=2))
        l4_pool = ctx.enter_context(tc.tile_pool(name="l4", bufs=2))

        for pbi, pb in enumerate((0, 64)):
            for b in range(BPC):
                rt = rts[b]
                sig_t = sig_pool.tile([OUT_F, HALF], bf16)
                for c in range(3):
                    pzt = pz.tile([OUT_F, 1536], f32, tag="z")
                    co = c * 1536
                    for m0 in range(0, 1536, MM_FREE):
                        mw = min(MM_FREE, 1536 - m0)
                        nc.tensor.matmul(
                            pzt[:, m0:m0 + mw],
                            lhsT=w2_sb[pb:pb + OP_D + 1, :],
                            rhs=rt[pb:pb + OP_D + 1, co + m0:co + m0 + mw],
                            start=True, stop=True)
                    nc.scalar.activation(out=sig_t[:, co:co + 1536],
                                         in_=pzt[:], func=SIG,
                                         bias=attnb_sb, scale=1.0)

                st_b = stbf_sb[:, b * N:(b + 1) * N]
                st_bcast = bass.AP(tensor=st_b.tensor, offset=st_b.offset,
                                   ap=[st_b.ap[0], [0, NIH], st_b.ap[1]])
                prod = prod_pool.tile([OUT_F, HALF], bf16)
                nc.vector.tensor_tensor(out=prod[:], in0=sig_t[:],
                                        in1=st_bcast, op=MUL)
                # tree: 96 -> 48 (DVE) -> 24 (Pool) -> 12 (Pool) -> 6 (Pool) -> 1 (DVE)
                l1 = l1_pool.tile([OUT_F, NIH * 48], bf16)
                nc.vector.tensor_tensor(
                    out=l1[:],
                    in0=sub_ap(prod[:], 0, [[96, NIH], [1, 48]]),
                    in1=sub_ap(prod[:], 48, [[96, NIH], [1, 48]]), op=ADD)
                l2 = l2_pool.tile([OUT_F, NIH * 24], bf16)
                nc.vector.tensor_tensor(
                    out=l2[:],
                    in0=sub_ap(l1[:], 0, [[48, NIH], [1, 24]]),
                    in1=sub_ap(l1[:], 24, [[48, NIH], [1, 24]]), op=ADD)
                l3 = l2_pool.tile([OUT_F, NIH * 12], bf16, tag="l3")
                nc.vector.tensor_tensor(
                    out=l3[:],
                    in0=sub_ap(l2[:], 0, [[24, NIH], [1, 12]]),
                    in1=sub_ap(l2[:], 12, [[24, NIH], [1, 12]]), op=ADD)
                l4 = l4_pool.tile([OUT_F, NIH * 6], bf16)
                nc.gpsimd.tensor_tensor(
                    out=l4[:],
                    in0=sub_ap(l3[:], 0, [[12, NIH], [1, 6]]),
                    in1=sub_ap(l3[:], 6, [[12, NIH], [1, 6]]), op=ADD)
                cb = b * N + pbi * NIH
                nc.vector.tensor_reduce(out=out_fin[:, cb:cb + NIH],
                                        in_=sub_ap(l4[:], 0, [[6, NIH], [1, 6]]),
                                        axis=mybir.AxisListType.X, op=ADD)

        # one bulk skip/identity add over all batches
        nc.vector.tensor_tensor(out=out_fin[:], in0=out_fin[:], in1=s1_sb[:],
                                op=ADD)

        # ---------------- output transpose + store ----------------
        outp = ctx.enter_context(tc.tile_pool(name="outp", bufs=2))
        for c in range(6):
            pt = ptr.tile([128, 512], f32, tag="pt")
            nc.tensor.transpose(pt[:, 0:128], out_fin[:, c * 128:(c + 1) * 128],
                                ident_sb)
            ot = outp.tile([128, 128], f32)
            nc.vector.tensor_copy(out=ot[:], in_=pt[:, 0:128])
            nc.sync.dma_start(out=p_out[c * 128:(c + 1) * 128, :], in_=ot[:])

    nc.finalize()
    return nc


def _get_nc():
    if "nc" not in _CACHE:
        _CACHE["nc"] = _build_nc()
    return _CACHE["nc"]


def marshal_core(inputs, adj, op_emb, weight, attn_w, attn_b, self_op_emb, core):
    """Build the in_map for one core (layout/dtype marshaling + mask logits)."""
    import ml_dtypes
    bfloat16 = ml_dtypes.bfloat16

    sl = slice(core * BPC, (core + 1) * BPC)
    op_sh = np.array(op_emb[sl], np.float32)              # [BPC, N, N, OP_D]
    idx = np.arange(N)
    op_sh[:, idx, idx, :] = np.asarray(self_op_emb, np.float32)
    op_t = op_sh.transpose(0, 3, 1, 2)                    # [BPC, OP_D, N(i), N(j)]
    adj_sh = np.asarray(adj[sl]).astype(np.int32)         # [BPC, N, N]
    eye = np.eye(N, dtype=np.float32)
    adjp = adj_sh.astype(np.float32) + eye
    m2 = np.where(adjp <= 1.0, np.float32(NEG), np.float32(0.0))  # [BPC, N, N]

    op4 = np.empty((BPC, 2, OP_D + 1, HALF), bfloat16)
    op4[:, :, :OP_D, :] = op_t.reshape(BPC, OP_D, 2, HALF).transpose(
        0, 2, 1, 3).astype(bfloat16)
    op4[:, :, OP_D, :] = m2.reshape(BPC, 2, HALF).astype(bfloat16)

    adjs = np.ascontiguousarray(adj_sh.transpose(2, 0, 1))  # [j, b, i]
    inpt = np.ascontiguousarray(
        np.asarray(inputs[sl], np.float32).reshape(BPC * N, IN_F).T)

    w2 = np.zeros((128, 128), np.float32)
    w2[0:OP_D] = attn_w
    w2[OP_D] = 1.0
    w2[64:64 + OP_D] = attn_w
    w2[64 + OP_D] = 1.0

    pbf = np.zeros((128, 1024), bfloat16)
    pbf[:, 0:128] = w2.astype(bfloat16)
    pbf[:, 128:256] = np.asarray(weight, np.float32).astype(bfloat16)
    pbf[:, 256:1024] = inpt.astype(bfloat16)
    pf32 = np.zeros((128, 321), np.float32)
    pf32[:, 0:128] = np.eye(128, dtype=np.float32)
    pf32[:, 128] = np.asarray(attn_b, np.float32)
    pf32[0:N, 129:225] = 1.0 - eye
    pf32[0:N, 225:321] = eye

    return {
        "op4": op4,
        "adjs": adjs,
        "pbf": pbf,
        "pf32": pf32,
    }


def _ensure_ntff_hook():
    """Provide antenv.axon_hooks if the image lacks it (NTFF timing under axon)."""
    import sys as _sys

    try:
        from antenv.axon_hooks import get_axon_ntff_profile_hook  # noqa: F401
        return
    except ImportError:
        pass

    import contextlib
    import ctypes
    import types

    so_path = "/opt/axon/libaxon_pjrt.so"
    try:
        lib = ctypes.CDLL(so_path)
    except OSError:
        lib = None
    if lib is None or not hasattr(lib, "axon_start_nrt_profile"):
        hook = None
    else:
        lib.axon_start_nrt_profile.argtypes = [
            ctypes.POINTER(ctypes.c_int64), ctypes.c_size_t]
        lib.axon_start_nrt_profile.restype = ctypes.c_int64
        lib.axon_stop_nrt_profile.argtypes = [ctypes.c_char_p]
        lib.axon_stop_nrt_profile.restype = ctypes.c_int64

        @contextlib.contextmanager
        def hook(output_dir, device_ids):
            import jax
            jax.devices()
            if device_ids:
                ids = (ctypes.c_int64 * len(device_ids))(*device_ids)
                rc = lib.axon_start_nrt_profile(ids, len(device_ids))
            else:
                rc = lib.axon_start_nrt_profile(None, 0)
            if rc != 0:
                raise RuntimeError(f"axon_start_nrt_profile rc={rc}")
            try:
                yield
            finally:
                n = lib.axon_stop_nrt_profile(str(output_dir).encode())
                print(f"ntff profile: {n} file(s) written to {output_dir}")

    mod = types.ModuleType("antenv.axon_hooks")
    _state = {"hook": hook}
    mod.get_axon_ntff_profile_hook = lambda: _state["hook"]

    def _set(h):
        _state["hook"] = h

    mod.set_axon_ntff_profile_hook = _set
    _sys.modules["antenv.axon_hooks"] = mod


def run(inputs, adj, op_emb, weight, attn_w, attn_b, self_op_emb, trace=False):
    if trace:
        _ensure_ntff_hook()
    from concourse.bass_utils import run_bass_kernel_spmd

    nc = _get_nc()
    in_maps = [
        marshal_core(inputs, adj, op_emb, weight, attn_w, attn_b, self_op_emb, c)
        for c in range(NCORES)
    ]
    res = run_bass_kernel_spmd(nc, in_maps, core_ids=list(range(NCORES)), trace=trace)
    out = np.concatenate(
        [res.results[c]["out"].reshape(BPC, N, OUT_F) for c in range(NCORES)], axis=0)
    return np.ascontiguousarray(out, np.float32), res


def kernel(inputs, adj, op_emb, weight, attn_w, attn_b, self_op_emb):
    out, _ = run(inputs, adj, op_emb, weight, attn_w, attn_b, self_op_emb, trace=False)
    return out
